# revision 10
# baseline (speedup 1.0000x reference)
"""PillarFeatureNet on 8 TRN2 NeuronCores, data-parallel over pillars.

Decomposition: the 9 augmented features are affine in the 4 raw voxel
features V and a 5-dim pillar vector q = [mean3, cx, cy]:
    x[r, c] = A_c . V_r + B_c . q_i            (valid points; 0 when masked)
BN batch stats therefore reduce to small moment matrices:
    sum x   = Wext . [SV | w1],   Wext = [A | B]  [64, 9]
    sum x^2 = Wext_c^T Syy Wext_c,  Syy = [[G, C], [C^T, H]]
with G = sum_valid V V^T (per-point), C = sum_i Mv_i q_i^T,
H = sum_i np_i q_i q_i^T, SV = sum Mv, w1 = sum np_i q_i (pillar-level).
The per-point work is one K=5 matmul ([V; invalid] with a -BIG indicator
row) and a masked max over each pillar's 32 points; BN+relu commute with
the max via sign-folded weights (At = sign(gamma)*A).

Phase 1 NEFF: flat-layout stats -> packed partials pk [16,16] per core
(host sums the 8 tiny partials - stand-in for the BN stat all-reduce).
Phase 2 NEFF: K=5 matmul 8-way tile-packed, segmented reduce_max, pillar
bias matmul, tail relu(|s|*(maxu+pb')+b).
"""

import numpy as np

VOXEL_SIZE = (0.1, 0.1, 5.0)
PC_RANGE = (-20.0, -20.0, -2.0, 20.0, 20.0, 3.0)
BN_EPS = 1e-3
N_CORES = 8
N_FULL = 60000
P = 32
OUT = 64
N_PER = N_FULL // N_CORES          # 7500
NPAD = 7680                        # 128 * 60
SLOTS = NPAD // 128                # 60
R = NPAD * P                       # 245760
BIG = 1.0e5
NP_TOTAL = float(N_FULL * P)


def _build_p1(bass, tile, mybir, bass_isa):
    dt = mybir.dt
    f32 = dt.float32
    op = mybir.AluOpType
    nc = bass.Bacc(num_devices=N_CORES)

    vflat = nc.dram_tensor("vflat", [128, SLOTS * P * 4], f32, kind="ExternalInput")
    npf = nc.dram_tensor("npf", [128, SLOTS], f32, kind="ExternalInput")
    cxy = nc.dram_tensor("cxy", [128, SLOTS * 2], f32, kind="ExternalInput")
    pko = nc.dram_tensor("pk", [16, 32], f32, kind="ExternalOutput")
    qo = nc.dram_tensor("qf", [5, NPAD], f32, kind="ExternalOutput")
    indo = nc.dram_tensor("indf", [128, SLOTS * P], f32, kind="ExternalOutput")

    with tile.TileContext(nc) as tc:
        with (
            tc.tile_pool(name="big", bufs=1) as big,
            tc.tile_pool(name="small", bufs=1) as small,
            tc.tile_pool(name="ps", bufs=2, space="PSUM") as psp,
        ):
            ftile = big.tile([128, SLOTS, P, 4], f32, tag="ftile")
            nc.sync.dma_start(ftile, vflat.rearrange(
                "p (s t f) -> p s t f", t=P, f=4))
            nptile = small.tile([128, SLOTS], f32)
            nc.sync.dma_start(nptile, npf[:, :])
            ctile = small.tile([128, SLOTS, 2], f32)
            nc.sync.dma_start(ctile, cxy.rearrange("p (s t) -> p s t", t=2))

            iot_i = small.tile([128, P], dt.int32)
            nc.gpsimd.iota(iot_i, pattern=[[1, P]], base=0, channel_multiplier=0)
            iot = small.tile([128, P], f32)
            nc.vector.tensor_copy(iot, iot_i)

            mtile = big.tile([128, SLOTS, P], f32, tag="mtile")
            nc.vector.tensor_tensor(
                out=mtile,
                in0=iot.rearrange("p (o t) -> p o t", o=1).broadcast_to([128, SLOTS, P]),
                in1=nptile.rearrange("p (s o) -> p s o", o=1).broadcast_to([128, SLOTS, P]),
                op=op.is_lt,
            )
            indt = big.tile([128, SLOTS, P], f32, tag="indt")
            nc.vector.tensor_scalar(out=indt, in0=mtile, scalar1=-1.0,
                                    scalar2=1.0, op0=op.mult, op1=op.add)
            nc.sync.dma_start(indo.rearrange("p (s t) -> p s t", t=P), indt)

            vmt = big.tile([128, SLOTS, P, 4], f32, tag="vmt")
            nc.vector.tensor_tensor(
                out=vmt, in0=ftile,
                in1=mtile.rearrange("p s (t o) -> p s t o", o=1).broadcast_to(
                    [128, SLOTS, P, 4]),
                op=op.mult,
            )

            mvt = small.tile([128, SLOTS, 4], f32)
            nc.vector.reduce_sum(mvt, vmt.rearrange("p s t f -> p s f t"),
                                 axis=mybir.AxisListType.X)
            mat = small.tile([128, SLOTS, 4], f32)
            nc.vector.reduce_sum(mat, ftile.rearrange("p s t f -> p s f t"),
                                 axis=mybir.AxisListType.X)

            ZS = 16
            ZSL = 64
            zmt = big.tile([128, ZSL, ZS], f32, tag="zmt")
            nc.vector.memset(zmt, 0.0)
            clipnp = small.tile([128, SLOTS], f32)
            nc.vector.tensor_scalar(out=clipnp, in0=nptile, scalar1=1.0,
                                    scalar2=None, op0=op.max)
            rnp = small.tile([128, SLOTS], f32)
            nc.vector.reciprocal(rnp, clipnp)
            nc.vector.tensor_copy(zmt[:, :SLOTS, 0:4], mvt)
            nc.vector.tensor_tensor(
                out=zmt[:, :SLOTS, 4:7], in0=mat[:, :, 0:3],
                in1=rnp.rearrange("p (s o) -> p s o", o=1).broadcast_to([128, SLOTS, 3]),
                op=op.mult)
            nc.vector.tensor_scalar(
                out=zmt[:, :SLOTS, 7:9], in0=ctile,
                scalar1=VOXEL_SIZE[0], scalar2=VOXEL_SIZE[0] / 2 + PC_RANGE[0],
                op0=op.mult, op1=op.add)
            nc.vector.tensor_tensor(
                out=zmt[:, :SLOTS, 9:14], in0=zmt[:, :SLOTS, 4:9],
                in1=nptile.rearrange("p (s o) -> p s o", o=1).broadcast_to(
                    [128, SLOTS, 5]),
                op=op.mult)
            nc.vector.memset(zmt[:, :SLOTS, 14:15], 1.0)
            qov = qo.rearrange("f (j q) -> f q j", q=128)
            for f in range(5):
                nc.sync.dma_start(qov[f:f + 1, :, :], zmt[:, :SLOTS, 4 + f:5 + f])

            psZ = psp.tile([128, 512], f32, tag="ps")
            zv = zmt.rearrange("p s z -> p (s z)")
            for w in range(8):
                nc.tensor.matmul(psZ[:, 0:128], lhsT=zv[:, 128 * w:128 * (w + 1)],
                                 rhs=zv[:, 128 * w:128 * (w + 1)],
                                 start=(w == 0), stop=(w == 7))
            zS = small.tile([128, 128], f32)
            nc.scalar.copy(zS, psZ[:, 0:128])

            psG = psp.tile([128, 512], f32, tag="ps")
            vmv = vmt.rearrange("p s t f -> p (s t f)")
            fv = ftile.rearrange("p s t f -> p (s t f)")
            for w in range(SLOTS):
                nc.tensor.matmul(psG[:, 0:128], lhsT=vmv[:, 128 * w:128 * (w + 1)],
                                 rhs=fv[:, 128 * w:128 * (w + 1)],
                                 start=(w == 0), stop=(w == SLOTS - 1))
            gS = small.tile([128, 128], f32)
            nc.scalar.copy(gS, psG[:, 0:128])

            zdiag = small.tile([16, 8, 16], f32)
            for g in range(8):
                nc.sync.dma_start(zdiag[:, g, :],
                                  zS[16 * g:16 * (g + 1), 16 * g:16 * (g + 1)])
            Sz = small.tile([16, 16], f32)
            nc.vector.reduce_sum(Sz, zdiag.rearrange("p g z -> p z g"),
                                 axis=mybir.AxisListType.X)
            gdiag = small.tile([4, 32, 4], f32)
            for g in range(32):
                nc.sync.dma_start(gdiag[:, g, :],
                                  gS[4 * g:4 * (g + 1), 4 * g:4 * (g + 1)])
            G4 = small.tile([4, 4], f32)
            nc.vector.reduce_sum(G4, gdiag.rearrange("p g z -> p z g"),
                                 axis=mybir.AxisListType.X)

            pk = small.tile([16, 32], f32)
            nc.vector.memset(pk, 0.0)
            nc.vector.tensor_copy(pk[:, 0:16], Sz)
            nc.vector.tensor_copy(pk[0:4, 16:20], G4)
            nc.sync.dma_start(pko[:, :], pk)
    nc.compile()
    return nc


def _build_p2(bass, tile, mybir, bass_isa):
    dt = mybir.dt
    f32 = dt.float32
    op = mybir.AluOpType
    nc = bass.Bacc(num_devices=N_CORES)

    vt = nc.dram_tensor("vt", [4, R], f32, kind="ExternalInput")
    qf = nc.dram_tensor("qf", [5, NPAD], f32, kind="ExternalInput")
    indf = nc.dram_tensor("indf", [128, SLOTS * P], f32, kind="ExternalInput")
    syyd = nc.dram_tensor("syy", [9, 9], f32, kind="ExternalInput")
    momd = nc.dram_tensor("mom", [9, 1], f32, kind="ExternalInput")
    a5 = nc.dram_tensor("a5", [5, OUT], f32, kind="ExternalInput")
    bt5 = nc.dram_tensor("bt5", [5, OUT], f32, kind="ExternalInput")
    wext = nc.dram_tensor("wext", [9, OUT], f32, kind="ExternalInput")
    grow = nc.dram_tensor("grow", [1, OUT], f32, kind="ExternalInput")
    brow = nc.dram_tensor("brow", [1, OUT], f32, kind="ExternalInput")
    srow = nc.dram_tensor("srow", [1, OUT], f32, kind="ExternalInput")
    outd = nc.dram_tensor("out", [128, NPAD // 2], f32, kind="ExternalOutput")

    with tile.TileContext(nc) as tc:
        with (
            tc.tile_pool(name="big", bufs=1) as big,
            tc.tile_pool(name="work", bufs=2) as work,
            tc.tile_pool(name="small", bufs=1) as small,
            tc.tile_pool(name="rhs", bufs=3) as rhsp,
            tc.tile_pool(name="ps", bufs=2, space="PSUM") as psp,
        ):
            # ---- stats finalization from summed partials ----
            syys = small.tile([9, 9], f32)
            nc.sync.dma_start(syys, syyd[:, :])
            momt = small.tile([9, 1], f32)
            nc.sync.dma_start(momt, momd[:, :])
            wxs = small.tile([9, OUT], f32)
            nc.sync.dma_start(wxs, wext[:, :])

            psT = psp.tile([128, 2048], f32, tag="ps")
            nc.tensor.matmul(psT[0:9, 0:OUT], lhsT=syys, rhs=wxs,
                             start=True, stop=True)
            nc.tensor.matmul(psT[0:1, 512:512 + OUT], lhsT=momt, rhs=wxs,
                             start=True, stop=True, tile_position=(0, 0))

            tws = small.tile([9, OUT], f32)
            nc.vector.tensor_tensor(out=tws, in0=psT[0:9, 0:OUT], in1=wxs,
                                    op=op.mult)
            tsum = small.tile([9, OUT], f32)
            nc.gpsimd.partition_all_reduce(tsum, tws, channels=9,
                                           reduce_op=bass_isa.ReduceOp.add)

            grow_s = small.tile([1, OUT], f32)
            nc.sync.dma_start(grow_s, grow[:, :])
            brow_s = small.tile([1, OUT], f32)
            nc.sync.dma_start(brow_s, brow[:, :])
            srow_s = small.tile([1, OUT], f32)
            nc.sync.dma_start(srow_s, srow[:, :])

            mean = small.tile([1, OUT], f32)
            nc.vector.tensor_scalar(out=mean, in0=psT[0:1, 512:512 + OUT],
                                    scalar1=1.0 / NP_TOTAL, scalar2=None,
                                    op0=op.mult)
            var = small.tile([1, OUT], f32)
            nc.vector.scalar_tensor_tensor(
                out=var, in0=mean, scalar=-1.0, in1=mean,
                op0=op.mult, op1=op.mult)
            nc.vector.scalar_tensor_tensor(
                out=var, in0=tsum[0:1, :], scalar=1.0 / NP_TOTAL, in1=var,
                op0=op.mult, op1=op.add)
            std = small.tile([1, OUT], f32)
            nc.vector.tensor_scalar(out=std, in0=var, scalar1=BN_EPS,
                                    scalar2=None, op0=op.add)
            nc.scalar.sqrt(std, std)
            rstd = small.tile([1, OUT], f32)
            nc.vector.reciprocal(rstd, std)
            srow_t = small.tile([1, OUT], f32)
            nc.vector.tensor_tensor(out=srow_t, in0=grow_s, in1=rstd, op=op.mult)
            sabs = small.tile([1, OUT], f32)
            nc.vector.tensor_tensor(out=sabs, in0=srow_t, in1=srow_s, op=op.mult)
            brow_t = small.tile([1, OUT], f32)
            nc.vector.scalar_tensor_tensor(
                out=brow_t, in0=mean, scalar=-1.0, in1=srow_t,
                op0=op.mult, op1=op.mult)
            nc.vector.tensor_tensor(out=brow_t, in0=brow_t, in1=brow_s, op=op.add)

            sc128 = small.tile([128, 1], f32)
            bc128 = small.tile([128, 1], f32)
            nc.sync.dma_start(sc128[0:64, :], sabs[:, :])
            nc.sync.dma_start(sc128[64:128, :], sabs[:, :])
            nc.sync.dma_start(bc128[0:64, :], brow_t[:, :])
            nc.sync.dma_start(bc128[64:128, :], brow_t[:, :])

            # ---- main matmul + segmented max ----
            indt = big.tile([128, SLOTS, P], f32, tag="indt")
            nc.sync.dma_start(indt, indf.rearrange("p (s t) -> p s t", t=P))
            a5s = small.tile([128, OUT], f32)
            for i in range(4):
                nc.sync.dma_start(a5s[32 * i:32 * i + 5, :], a5[:, :])

            maxu = big.tile([128, NPAD // 2], f32, tag="maxu")
            NSB = R // 4096
            for s in range(NSB):
                rt = rhsp.tile([128, 1024], f32, tag="rt")
                off = 4096 * s
                for i in range(4):
                    nc.sync.dma_start(
                        rt[32 * i:32 * i + 4, :],
                        vt[:, off + 1024 * i: off + 1024 * (i + 1)])
                    nc.sync.dma_start(rt[32 * i + 4:32 * i + 5, :],
                                      indt[32 * i:32 * (i + 1), s:s + 1, :])
                pst = psp.tile([128, 2048], f32, tag="ps")
                for i in range(4):
                    for j in range(2):
                        nc.tensor.matmul(
                            pst[64 * j:64 * (j + 1), 512 * i:512 * i + 512],
                            lhsT=a5s[32 * i:32 * i + 5, :],
                            rhs=rt[32 * i:32 * i + 5, 512 * j:512 * (j + 1)],
                            start=True, stop=True,
                            tile_position=(32 * i, 64 * j))
                nc.vector.reduce_max(
                    maxu[:, 64 * s:64 * (s + 1)],
                    pst.rearrange("p (i a t) -> p i a t", a=16, t=P),
                    axis=mybir.AxisListType.X)

            # ---- pillar bias + tail ----
            qT = big.tile([16, NPAD], f32, tag="qT")
            nc.sync.dma_start(qT[0:5, :], qf[:, :])
            bts = small.tile([16, OUT], f32)
            nc.sync.dma_start(bts[0:5, :], bt5[:, :])

            qa = qT[0:5, :].rearrange("f (b t) -> f b t", t=P)
            for k in range(2):
                pb = psp.tile([128, 2048], f32, tag="ps")
                for c in range(4):
                    bsl = slice(120 * k + 30 * c, 120 * k + 30 * (c + 1))
                    nc.tensor.matmul(
                        pb[0:64, 512 * c:512 * c + 480],
                        lhsT=bts[0:5, :],
                        rhs=qa[:, bsl, 0:16],
                        start=True, stop=True, tile_position=(0, 0))
                    nc.tensor.matmul(
                        pb[64:128, 512 * c:512 * c + 480],
                        lhsT=bts[0:5, :],
                        rhs=qa[:, bsl, 16:32],
                        start=True, stop=True, tile_position=(0, 64))
                tt = work.tile([128, 4, 480], f32, tag="tt")
                nc.vector.tensor_tensor(
                    out=tt,
                    in0=maxu[:, 1920 * k:1920 * (k + 1)].rearrange(
                        "p (c x) -> p c x", x=480),
                    in1=pb.rearrange("p (c x) -> p c x", x=512)[:, :, 0:480],
                    op=op.add)
                otile = work.tile([128, 1920], f32, tag="otile")
                nc.scalar.activation(
                    otile, tt.rearrange("p c x -> p (c x)"),
                    mybir.ActivationFunctionType.Relu,
                    bias=bc128[:, 0:1], scale=sc128[:, 0:1])
                nc.sync.dma_start(outd[:, 1920 * k:1920 * (k + 1)], otile)
    nc.compile()
    return nc


_CACHE = {}


def _get_programs():
    if "p" not in _CACHE:
        import concourse.bacc as bass
        import concourse.tile as tile
        import concourse.mybir as mybir
        import concourse.bass_isa as bass_isa
        _CACHE["p"] = (
            _build_p1(bass, tile, mybir, bass_isa),
            _build_p2(bass, tile, mybir, bass_isa),
        )
    return _CACHE["p"]


def kernel(voxels, num_points, coors, W, gamma, beta):
    import os
    os.environ["BASS_NEVER_TRACE"] = "1"  # no axon NTFF hook in this container
    from concourse.bass_utils import run_bass_kernel_spmd

    W = np.asarray(W, np.float32)
    gamma = np.asarray(gamma, np.float32)
    beta = np.asarray(beta, np.float32)
    voxels = np.asarray(voxels, np.float32)
    npi = np.asarray(num_points, np.int32)
    coi = np.asarray(coors, np.int32)

    A = np.stack([W[:, 0] + W[:, 4] + W[:, 7], W[:, 1] + W[:, 5] + W[:, 8],
                  W[:, 2] + W[:, 6], W[:, 3]], axis=1)
    B = np.concatenate([-W[:, 4:7], -W[:, 7:9]], axis=1)
    sgn = np.where(gamma >= 0, 1.0, -1.0).astype(np.float32)
    a5 = np.concatenate([(A * sgn[:, None]).T,
                         np.full((1, OUT), -BIG, np.float32)], 0).copy()
    bt5 = np.ascontiguousarray((B * sgn[:, None]).T)
    wext = np.ascontiguousarray(np.concatenate([A, B], axis=1).T)

    nc1, nc2 = _get_programs()

    in1, vts = [], []
    for c in range(N_CORES):
        sl = slice(c * N_PER, (c + 1) * N_PER)
        vox = np.zeros((NPAD, P, 4), np.float32)
        vox[:N_PER] = voxels[sl]
        npv = np.zeros((NPAD,), np.float32)
        npv[:N_PER] = npi[sl]
        co = np.zeros((NPAD, 2), np.float32)
        co[:N_PER] = coi[sl, 1:3]
        vts.append(np.ascontiguousarray(vox.reshape(R, 4).T))
        in1.append({
            "vflat": np.ascontiguousarray(
                vox.reshape(SLOTS, 128, P * 4).transpose(1, 0, 2)
            ).reshape(128, -1),
            "npf": np.ascontiguousarray(npv.reshape(SLOTS, 128).T),
            "cxy": np.ascontiguousarray(
                co.reshape(SLOTS, 128, 2).transpose(1, 0, 2)).reshape(128, -1),
        })

    r1 = run_bass_kernel_spmd(nc1, in1, core_ids=list(range(N_CORES)))
    if r1.exec_time_ns:
        print(f"HW exec time p1: {r1.exec_time_ns} ns; trace: "
              f"{r1.instructions_and_trace[1] if r1.instructions_and_trace else None}")
    pka = np.sum([r.get("pk") for r in r1.results], axis=0).astype(np.float32)
    Sz = pka[:, 0:16]
    G = pka[0:4, 16:20]
    Syy = np.block([[G, Sz[0:4, 4:9]], [Sz[0:4, 4:9].T, Sz[4:9, 9:14]]])
    Syy = Syy.astype(np.float32)
    mom = np.concatenate([Sz[14, 0:4], Sz[14, 9:14]]).astype(np.float32)[:, None]

    in2 = []
    for c in range(N_CORES):
        in2.append({
            "vt": vts[c], "qf": r1.results[c]["qf"],
            "indf": r1.results[c]["indf"], "syy": Syy, "mom": mom,
            "a5": a5, "bt5": bt5, "wext": wext,
            "grow": gamma[None, :].copy(), "brow": beta[None, :].copy(),
            "srow": sgn[None, :].copy(),
        })
    r2 = run_bass_kernel_spmd(nc2, in2, core_ids=list(range(N_CORES)))
    if r2.exec_time_ns:
        print(f"HW exec time p2: {r2.exec_time_ns} ns; trace: "
              f"{r2.instructions_and_trace[1] if r2.instructions_and_trace else None}")
        print(f"HW exec time: {(r1.exec_time_ns or 0) + r2.exec_time_ns} ns")

    out = np.empty((N_FULL, OUT), np.float32)
    for c in range(N_CORES):
        o = r2.results[c]["out"]
        full = o.reshape(2, OUT, NPAD // 32, 16).transpose(2, 0, 3, 1) \
                .reshape(NPAD, OUT)
        out[c * N_PER:(c + 1) * N_PER] = full[:N_PER]
    return out


# revision 12
# speedup vs baseline: 1.0198x; 1.0198x over previous
"""PillarFeatureNet on 8 TRN2 NeuronCores, data-parallel over pillars.

Decomposition: the 9 augmented features are affine in the 4 raw voxel
features V and a 5-dim pillar vector q = [mean3, cx, cy]:
    x[r, c] = A_c . V_r + B_c . q_i            (valid points; 0 when masked)
BN batch stats therefore reduce to small moment matrices:
    sum x   = Wext . [SV | w1],   Wext = [A | B]  [64, 9]
    sum x^2 = Wext_c^T Syy Wext_c,  Syy = [[G, C], [C^T, H]]
with G = sum_valid V V^T (per-point), C = sum_i Mv_i q_i^T,
H = sum_i np_i q_i q_i^T, SV = sum Mv, w1 = sum np_i q_i (pillar-level).
The per-point work is one K=5 matmul ([V; invalid] with a -BIG indicator
row) and a masked max over each pillar's 32 points; BN+relu commute with
the max via sign-folded weights (At = sign(gamma)*A).

Phase 1 NEFF: flat-layout stats -> packed partials pk [16,16] per core
(host sums the 8 tiny partials - stand-in for the BN stat all-reduce).
Phase 2 NEFF: K=5 matmul 8-way tile-packed, segmented reduce_max, pillar
bias matmul, tail relu(|s|*(maxu+pb')+b).
"""

import numpy as np

VOXEL_SIZE = (0.1, 0.1, 5.0)
PC_RANGE = (-20.0, -20.0, -2.0, 20.0, 20.0, 3.0)
BN_EPS = 1e-3
N_CORES = 8
N_FULL = 60000
P = 32
OUT = 64
N_PER = N_FULL // N_CORES          # 7500
NPAD = 7680                        # 128 * 60
SLOTS = NPAD // 128                # 60
R = NPAD * P                       # 245760
BIG = 1.0e5
NP_TOTAL = float(N_FULL * P)


def _build_p1(bass, tile, mybir, bass_isa):
    dt = mybir.dt
    f32 = dt.float32
    op = mybir.AluOpType
    nc = bass.Bacc(num_devices=N_CORES)

    vflat = nc.dram_tensor("vflat", [128, SLOTS * P * 4], f32, kind="ExternalInput")
    npf = nc.dram_tensor("npf", [128, SLOTS], f32, kind="ExternalInput")
    cxy = nc.dram_tensor("cxy", [128, SLOTS * 2], f32, kind="ExternalInput")
    pko = nc.dram_tensor("pk", [16, 32], f32, kind="ExternalOutput")
    qo = nc.dram_tensor("qf", [5, NPAD], f32, kind="ExternalOutput")
    indo = nc.dram_tensor("indf", [128, SLOTS * P], f32, kind="ExternalOutput")

    with tile.TileContext(nc) as tc:
        with (
            tc.tile_pool(name="big", bufs=1) as big,
            tc.tile_pool(name="small", bufs=1) as small,
            tc.tile_pool(name="ps", bufs=2, space="PSUM") as psp,
        ):
            ftile = big.tile([128, SLOTS, P, 4], f32, tag="ftile")
            nc.sync.dma_start(ftile, vflat.rearrange(
                "p (s t f) -> p s t f", t=P, f=4))
            nptile = small.tile([128, SLOTS], f32)
            nc.sync.dma_start(nptile, npf[:, :])
            ctile = small.tile([128, SLOTS, 2], f32)
            nc.sync.dma_start(ctile, cxy.rearrange("p (s t) -> p s t", t=2))

            iot_i = small.tile([128, P], dt.int32)
            nc.gpsimd.iota(iot_i, pattern=[[1, P]], base=0, channel_multiplier=0)
            iot = small.tile([128, P], f32)
            nc.vector.tensor_copy(iot, iot_i)

            mtile = big.tile([128, SLOTS, P], f32, tag="mtile")
            nc.vector.tensor_tensor(
                out=mtile,
                in0=iot.rearrange("p (o t) -> p o t", o=1).broadcast_to([128, SLOTS, P]),
                in1=nptile.rearrange("p (s o) -> p s o", o=1).broadcast_to([128, SLOTS, P]),
                op=op.is_lt,
            )
            indt = big.tile([128, SLOTS, P], f32, tag="indt")
            nc.vector.tensor_scalar(out=indt, in0=mtile, scalar1=-1.0,
                                    scalar2=1.0, op0=op.mult, op1=op.add)
            nc.sync.dma_start(indo.rearrange("p (s t) -> p s t", t=P), indt)

            vmt = big.tile([128, SLOTS, P, 4], f32, tag="vmt")
            nc.vector.tensor_tensor(
                out=vmt, in0=ftile,
                in1=mtile.rearrange("p s (t o) -> p s t o", o=1).broadcast_to(
                    [128, SLOTS, P, 4]),
                op=op.mult,
            )

            mvt = small.tile([128, SLOTS, 4], f32)
            nc.vector.reduce_sum(mvt, vmt.rearrange("p s t f -> p s f t"),
                                 axis=mybir.AxisListType.X)
            mat = small.tile([128, SLOTS, 4], f32)
            nc.vector.reduce_sum(mat, ftile.rearrange("p s t f -> p s f t"),
                                 axis=mybir.AxisListType.X)

            ZS = 16
            ZSL = 64
            zmt = big.tile([128, ZSL, ZS], f32, tag="zmt")
            nc.vector.memset(zmt, 0.0)
            clipnp = small.tile([128, SLOTS], f32)
            nc.vector.tensor_scalar(out=clipnp, in0=nptile, scalar1=1.0,
                                    scalar2=None, op0=op.max)
            rnp = small.tile([128, SLOTS], f32)
            nc.vector.reciprocal(rnp, clipnp)
            nc.vector.tensor_copy(zmt[:, :SLOTS, 0:4], mvt)
            nc.vector.tensor_tensor(
                out=zmt[:, :SLOTS, 4:7], in0=mat[:, :, 0:3],
                in1=rnp.rearrange("p (s o) -> p s o", o=1).broadcast_to([128, SLOTS, 3]),
                op=op.mult)
            nc.vector.tensor_scalar(
                out=zmt[:, :SLOTS, 7:9], in0=ctile,
                scalar1=VOXEL_SIZE[0], scalar2=VOXEL_SIZE[0] / 2 + PC_RANGE[0],
                op0=op.mult, op1=op.add)
            nc.vector.tensor_tensor(
                out=zmt[:, :SLOTS, 9:14], in0=zmt[:, :SLOTS, 4:9],
                in1=nptile.rearrange("p (s o) -> p s o", o=1).broadcast_to(
                    [128, SLOTS, 5]),
                op=op.mult)
            nc.vector.memset(zmt[:, :SLOTS, 14:15], 1.0)
            qov = qo.rearrange("f (j q) -> f q j", q=128)
            for f in range(5):
                nc.sync.dma_start(qov[f:f + 1, :, :], zmt[:, :SLOTS, 4 + f:5 + f])

            psZ = psp.tile([128, 512], f32, tag="ps")
            zv = zmt.rearrange("p s z -> p (s z)")
            for w in range(8):
                nc.tensor.matmul(psZ[:, 0:128], lhsT=zv[:, 128 * w:128 * (w + 1)],
                                 rhs=zv[:, 128 * w:128 * (w + 1)],
                                 start=(w == 0), stop=(w == 7))
            zS = small.tile([128, 128], f32)
            nc.scalar.copy(zS, psZ[:, 0:128])

            psG = psp.tile([128, 512], f32, tag="ps")
            vmv = vmt.rearrange("p s t f -> p (s t f)")
            fv = ftile.rearrange("p s t f -> p (s t f)")
            for w in range(SLOTS):
                nc.tensor.matmul(psG[:, 0:128], lhsT=vmv[:, 128 * w:128 * (w + 1)],
                                 rhs=fv[:, 128 * w:128 * (w + 1)],
                                 start=(w == 0), stop=(w == SLOTS - 1))
            gS = small.tile([128, 128], f32)
            nc.scalar.copy(gS, psG[:, 0:128])

            zdiag = small.tile([16, 8, 16], f32)
            for g in range(8):
                nc.sync.dma_start(zdiag[:, g, :],
                                  zS[16 * g:16 * (g + 1), 16 * g:16 * (g + 1)])
            Sz = small.tile([16, 16], f32)
            nc.vector.reduce_sum(Sz, zdiag.rearrange("p g z -> p z g"),
                                 axis=mybir.AxisListType.X)
            gdiag = small.tile([4, 32, 4], f32)
            for g in range(32):
                nc.sync.dma_start(gdiag[:, g, :],
                                  gS[4 * g:4 * (g + 1), 4 * g:4 * (g + 1)])
            G4 = small.tile([4, 4], f32)
            nc.vector.reduce_sum(G4, gdiag.rearrange("p g z -> p z g"),
                                 axis=mybir.AxisListType.X)

            pk = small.tile([16, 32], f32)
            nc.vector.memset(pk, 0.0)
            nc.vector.tensor_copy(pk[:, 0:16], Sz)
            nc.vector.tensor_copy(pk[0:4, 16:20], G4)
            nc.sync.dma_start(pko[:, :], pk)
    nc.compile()
    return nc


def _build_p2(bass, tile, mybir, bass_isa):
    dt = mybir.dt
    f32 = dt.float32
    op = mybir.AluOpType
    nc = bass.Bacc(num_devices=N_CORES)

    vt = nc.dram_tensor("vt", [4, R], f32, kind="ExternalInput")
    qf = nc.dram_tensor("qf", [5, NPAD], f32, kind="ExternalInput")
    indf = nc.dram_tensor("indf", [128, SLOTS * P], f32, kind="ExternalInput")
    syyd = nc.dram_tensor("syy", [9, 9], f32, kind="ExternalInput")
    momd = nc.dram_tensor("mom", [9, 1], f32, kind="ExternalInput")
    a5 = nc.dram_tensor("a5", [5, OUT], f32, kind="ExternalInput")
    bt5 = nc.dram_tensor("bt5", [5, OUT], f32, kind="ExternalInput")
    wext = nc.dram_tensor("wext", [9, OUT], f32, kind="ExternalInput")
    grow = nc.dram_tensor("grow", [1, OUT], f32, kind="ExternalInput")
    brow = nc.dram_tensor("brow", [1, OUT], f32, kind="ExternalInput")
    srow = nc.dram_tensor("srow", [1, OUT], f32, kind="ExternalInput")
    outd = nc.dram_tensor("out", [128, NPAD // 2], f32, kind="ExternalOutput")

    with tile.TileContext(nc) as tc:
        with (
            tc.tile_pool(name="big", bufs=1) as big,
            tc.tile_pool(name="work", bufs=2) as work,
            tc.tile_pool(name="small", bufs=1) as small,
            tc.tile_pool(name="rhs", bufs=3) as rhsp,
            tc.tile_pool(name="ps", bufs=2, space="PSUM") as psp,
        ):
            # ---- stats finalization from summed partials ----
            syys = small.tile([9, 9], f32)
            nc.sync.dma_start(syys, syyd[:, :])
            momt = small.tile([9, 1], f32)
            nc.sync.dma_start(momt, momd[:, :])
            wxs = small.tile([9, OUT], f32)
            nc.sync.dma_start(wxs, wext[:, :])

            psT = psp.tile([128, 2048], f32, tag="ps")
            nc.tensor.matmul(psT[0:9, 0:OUT], lhsT=syys, rhs=wxs,
                             start=True, stop=True)
            nc.tensor.matmul(psT[0:1, 512:512 + OUT], lhsT=momt, rhs=wxs,
                             start=True, stop=True, tile_position=(0, 0))

            tws = small.tile([9, OUT], f32)
            nc.vector.tensor_tensor(out=tws, in0=psT[0:9, 0:OUT], in1=wxs,
                                    op=op.mult)
            tsum = small.tile([9, OUT], f32)
            nc.gpsimd.partition_all_reduce(tsum, tws, channels=9,
                                           reduce_op=bass_isa.ReduceOp.add)

            grow_s = small.tile([1, OUT], f32)
            nc.sync.dma_start(grow_s, grow[:, :])
            brow_s = small.tile([1, OUT], f32)
            nc.sync.dma_start(brow_s, brow[:, :])
            srow_s = small.tile([1, OUT], f32)
            nc.sync.dma_start(srow_s, srow[:, :])

            mean = small.tile([1, OUT], f32)
            nc.vector.tensor_scalar(out=mean, in0=psT[0:1, 512:512 + OUT],
                                    scalar1=1.0 / NP_TOTAL, scalar2=None,
                                    op0=op.mult)
            var = small.tile([1, OUT], f32)
            nc.vector.scalar_tensor_tensor(
                out=var, in0=mean, scalar=-1.0, in1=mean,
                op0=op.mult, op1=op.mult)
            nc.vector.scalar_tensor_tensor(
                out=var, in0=tsum[0:1, :], scalar=1.0 / NP_TOTAL, in1=var,
                op0=op.mult, op1=op.add)
            std = small.tile([1, OUT], f32)
            nc.vector.tensor_scalar(out=std, in0=var, scalar1=BN_EPS,
                                    scalar2=None, op0=op.add)
            nc.scalar.sqrt(std, std)
            rstd = small.tile([1, OUT], f32)
            nc.vector.reciprocal(rstd, std)
            srow_t = small.tile([1, OUT], f32)
            nc.vector.tensor_tensor(out=srow_t, in0=grow_s, in1=rstd, op=op.mult)
            sabs = small.tile([1, OUT], f32)
            nc.vector.tensor_tensor(out=sabs, in0=srow_t, in1=srow_s, op=op.mult)
            brow_t = small.tile([1, OUT], f32)
            nc.vector.scalar_tensor_tensor(
                out=brow_t, in0=mean, scalar=-1.0, in1=srow_t,
                op0=op.mult, op1=op.mult)
            nc.vector.tensor_tensor(out=brow_t, in0=brow_t, in1=brow_s, op=op.add)

            sc128 = small.tile([128, 1], f32)
            bc128 = small.tile([128, 1], f32)
            nc.sync.dma_start(sc128[0:64, :], sabs[:, :])
            nc.sync.dma_start(sc128[64:128, :], sabs[:, :])
            nc.sync.dma_start(bc128[0:64, :], brow_t[:, :])
            nc.sync.dma_start(bc128[64:128, :], brow_t[:, :])

            # ---- main matmul + segmented max ----
            indt = big.tile([128, SLOTS, P], f32, tag="indt")
            nc.sync.dma_start(indt, indf.rearrange("p (s t) -> p s t", t=P))
            a5s = small.tile([128, OUT], f32)
            for i in range(4):
                nc.sync.dma_start(a5s[32 * i:32 * i + 5, :], a5[:, :])

            maxu = big.tile([128, NPAD // 2], f32, tag="maxu")
            NSB = R // 4096
            for s in range(NSB):
                rt = rhsp.tile([128, 1024], f32, tag="rt")
                off = 4096 * s
                for i in range(4):
                    nc.sync.dma_start(
                        rt[32 * i:32 * i + 4, :],
                        vt[:, off + 1024 * i: off + 1024 * (i + 1)])
                    nc.sync.dma_start(rt[32 * i + 4:32 * i + 5, :],
                                      indt[32 * i:32 * (i + 1), s:s + 1, :])
                pst = psp.tile([128, 2048], f32, tag="ps")
                for i in range(4):
                    for j in range(2):
                        nc.tensor.matmul(
                            pst[64 * j:64 * (j + 1), 512 * i:512 * i + 512],
                            lhsT=a5s[32 * i:32 * i + 5, :],
                            rhs=rt[32 * i:32 * i + 5, 512 * j:512 * (j + 1)],
                            start=True, stop=True,
                            tile_position=(32 * i, 64 * j))
                nc.vector.reduce_max(
                    maxu[:, 64 * s:64 * (s + 1)],
                    pst.rearrange("p (i a t) -> p i a t", a=16, t=P),
                    axis=mybir.AxisListType.X)

            # ---- pillar bias + tail ----
            qT = big.tile([16, NPAD], f32, tag="qT")
            nc.sync.dma_start(qT[0:5, :], qf[:, :])
            bts = small.tile([16, OUT], f32)
            nc.sync.dma_start(bts[0:5, :], bt5[:, :])

            qa = qT[0:5, :].rearrange("f (b t) -> f b t", t=P)
            for k in range(2):
                pb = psp.tile([128, 2048], f32, tag="ps")
                for c in range(4):
                    bsl = slice(120 * k + 30 * c, 120 * k + 30 * (c + 1))
                    nc.tensor.matmul(
                        pb[0:64, 512 * c:512 * c + 480],
                        lhsT=bts[0:5, :],
                        rhs=qa[:, bsl, 0:16],
                        start=True, stop=True, tile_position=(0, 0))
                    nc.tensor.matmul(
                        pb[64:128, 512 * c:512 * c + 480],
                        lhsT=bts[0:5, :],
                        rhs=qa[:, bsl, 16:32],
                        start=True, stop=True, tile_position=(0, 64))
                tt = work.tile([128, 4, 480], f32, tag="tt")
                nc.vector.tensor_tensor(
                    out=tt,
                    in0=maxu[:, 1920 * k:1920 * (k + 1)].rearrange(
                        "p (c x) -> p c x", x=480),
                    in1=pb.rearrange("p (c x) -> p c x", x=512)[:, :, 0:480],
                    op=op.add)
                otile = work.tile([128, 1920], f32, tag="otile")
                nc.scalar.activation(
                    otile, tt.rearrange("p c x -> p (c x)"),
                    mybir.ActivationFunctionType.Relu,
                    bias=bc128[:, 0:1], scale=sc128[:, 0:1])
                nc.sync.dma_start(outd[:, 1920 * k:1920 * (k + 1)], otile)
    nc.compile()
    return nc


_CACHE = {}


def _get_programs():
    if "p" not in _CACHE:
        import concourse.bacc as bass
        import concourse.tile as tile
        import concourse.mybir as mybir
        import concourse.bass_isa as bass_isa
        _CACHE["p"] = (
            _build_p1(bass, tile, mybir, bass_isa),
            _build_p2(bass, tile, mybir, bass_isa),
        )
    return _CACHE["p"]


def kernel(voxels, num_points, coors, W, gamma, beta):
    import os
    os.environ["BASS_NEVER_TRACE"] = "1"  # no axon NTFF hook in this container
    from concourse.bass_utils import run_bass_kernel_spmd

    W = np.asarray(W, np.float32)
    gamma = np.asarray(gamma, np.float32)
    beta = np.asarray(beta, np.float32)
    voxels = np.asarray(voxels, np.float32)
    npi = np.asarray(num_points, np.int32)
    coi = np.asarray(coors, np.int32)

    A = np.stack([W[:, 0] + W[:, 4] + W[:, 7], W[:, 1] + W[:, 5] + W[:, 8],
                  W[:, 2] + W[:, 6], W[:, 3]], axis=1)
    B = np.concatenate([-W[:, 4:7], -W[:, 7:9]], axis=1)
    sgn = np.where(gamma >= 0, 1.0, -1.0).astype(np.float32)
    a5 = np.concatenate([(A * sgn[:, None]).T,
                         np.full((1, OUT), -BIG, np.float32)], 0).copy()
    bt5 = np.ascontiguousarray((B * sgn[:, None]).T)
    wext = np.ascontiguousarray(np.concatenate([A, B], axis=1).T)

    nc1, nc2 = _get_programs()

    in1, vts = [], []
    for c in range(N_CORES):
        sl = slice(c * N_PER, (c + 1) * N_PER)
        vox = np.zeros((NPAD, P, 4), np.float32)
        vox[:N_PER] = voxels[sl]
        npv = np.zeros((NPAD,), np.float32)
        npv[:N_PER] = npi[sl]
        co = np.zeros((NPAD, 2), np.float32)
        co[:N_PER] = coi[sl, 1:3]
        vts.append(np.ascontiguousarray(vox.reshape(R, 4).T))
        in1.append({
            "vflat": np.ascontiguousarray(
                vox.reshape(SLOTS, 128, P * 4).transpose(1, 0, 2)
            ).reshape(128, -1),
            "npf": np.ascontiguousarray(npv.reshape(SLOTS, 128).T),
            "cxy": np.ascontiguousarray(
                co.reshape(SLOTS, 128, 2).transpose(1, 0, 2)).reshape(128, -1),
        })

    r1 = run_bass_kernel_spmd(nc1, in1, core_ids=list(range(N_CORES)))
    if r1.exec_time_ns:
        print(f"HW exec time p1: {r1.exec_time_ns} ns; trace: "
              f"{r1.instructions_and_trace[1] if r1.instructions_and_trace else None}")
    pka = np.sum([r.get("pk") for r in r1.results], axis=0).astype(np.float32)
    Sz = pka[:, 0:16]
    G = pka[0:4, 16:20]
    Syy = np.block([[G, Sz[0:4, 4:9]], [Sz[0:4, 4:9].T, Sz[4:9, 9:14]]])
    Syy = Syy.astype(np.float32)
    mom = np.concatenate([Sz[14, 0:4], Sz[14, 9:14]]).astype(np.float32)[:, None]

    in2 = []
    for c in range(N_CORES):
        in2.append({
            "vt": vts[c], "qf": r1.results[c]["qf"],
            "indf": r1.results[c]["indf"], "syy": Syy, "mom": mom,
            "a5": a5, "bt5": bt5, "wext": wext,
            "grow": gamma[None, :].copy(), "brow": beta[None, :].copy(),
            "srow": sgn[None, :].copy(),
        })
    r2 = run_bass_kernel_spmd(nc2, in2, core_ids=list(range(N_CORES)))
    if r2.exec_time_ns:
        print(f"HW exec time p2: {r2.exec_time_ns} ns; trace: "
              f"{r2.instructions_and_trace[1] if r2.instructions_and_trace else None}")
        print(f"HW exec time: {(r1.exec_time_ns or 0) + r2.exec_time_ns} ns")

    out = np.empty((N_FULL, OUT), np.float32)
    for c in range(N_CORES):
        o = r2.results[c]["out"]
        full = o.reshape(2, OUT, NPAD // 32, 16).transpose(2, 0, 3, 1) \
                .reshape(NPAD, OUT)
        out[c * N_PER:(c + 1) * N_PER] = full[:N_PER]
    return out
